# revision 59
# baseline (speedup 1.0000x reference)
# BertSelfAttention TRN2 Bass kernel.
#
# Full-input contract: kernel(**inputs) takes the unsharded tensors and
# returns the full [2, 2048, 1024] output. Internally shards across 8
# NeuronCores: core c handles batch c//4 and heads 4*(c%4) .. 4*(c%4)+3
# (data parallel over batch x tensor parallel over heads; no cross-core
# communication, host gathers).
#
# Per-core dataflow (fp16 matmul operands, fp32 PSUM accumulation):
#   X [2048,1024] f32 --PE transpose (f32r)--> copyback casts --> XT fp16
#   W [256,1024] fp16 (gpsimd casting DMA) --PE transpose--> WT [i,d]
#   QT/KT = WT.T @ XT -> [128 d, 2048 t] per head-pair; bias folded into the
#     PSUM->SBUF copyback via DVE tensor_scalar_add (per-partition scalar)
#   V natural [t, d] via XT.T @ WT_v, ones-row K=1 matmul adds bias;
#     stored per key-chunk/head with a 65th all-ones column
#   attention per (q-block 512, head-pair j, key-chunk kc):
#     S.T = K @ Q.T  2 row-packed matmuls -> psum [128 keys, 2x512]
#     P.T = exp(0.125*S.T + mask[key])  one ScalarE activation [128,1024]
#     C.T += V_aug.T @ P.T in half-bursts of 8 kc -> psum [65, 512] per head
#     (row 64 accumulates the softmax denominator); halves are combined in
#     an SBUF accumulator so only one pass's C psum is live at a time
#   drain: PE transpose C.T -> [128 q, 65]; DVE reciprocal, gpsimd scale.
#
# Emission order = per-engine execution order, so a greedy static scheduler
# interleaves the projection/transpose work with S/exp batches from many
# attention passes at once: the ScalarE exp stream (133us, the pacing
# engine) starts ~10us in and stays fed, while PE (the busiest engine,
# ~170us) never waits on DMA. The pt ring (exp outputs waiting for their
# C burst) bounds how far exp may run ahead of the C matmuls.

import numpy as np

import concourse.bass as bass
from concourse import bacc
import concourse.mybir as mybir
import concourse.tile as tile
from concourse.bass import ds, ts
from concourse.bass_utils import run_bass_kernel_spmd
from concourse.masks import make_identity

P = 128
L = 2048  # tokens per batch element
HF = 1024  # model width
DC = 256  # head dims per core (4 heads x 64)
F32 = mybir.dt.float32
F32R = mybir.dt.float32r
DT = mybir.dt.float16  # matmul operand dtype (PSUM accumulation stays fp32)
EXP = mybir.ActivationFunctionType.Exp

PT_RING = 36  # pt pool bufs: exp outputs outstanding before a C burst


def _emit(tc, x, wq, wk, wv, bq, bk, bv, mask, out, phases="all"):
    nc = tc.nc
    from contextlib import ExitStack

    with ExitStack() as es:
        consts = es.enter_context(tc.tile_pool(name="consts", bufs=1))
        wtp = es.enter_context(tc.tile_pool(name="wt", bufs=1))
        qkvp = es.enter_context(tc.tile_pool(name="qkv", bufs=1))
        wnat = es.enter_context(tc.tile_pool(name="wnat", bufs=1))
        xnat = es.enter_context(tc.tile_pool(name="xnat", bufs=1))
        ptp = es.enter_context(tc.tile_pool(name="ptp", bufs=PT_RING))
        csp = es.enter_context(tc.tile_pool(name="csp", bufs=3))
        rcpp = es.enter_context(tc.tile_pool(name="rcpp", bufs=2))
        drp = es.enter_context(tc.tile_pool(name="drp", bufs=2))
        outp = es.enter_context(tc.tile_pool(name="outp", bufs=2))
        # PSUM (8 banks of 2KB): bigp 2x2 (S tiles), ctps 2x1 (C.T
        # accumulators), tps4 2x1 (transpose staging + projection groups via
        # bitcast views)
        bigp = es.enter_context(tc.tile_pool(name="bigp", bufs=2, space="PSUM"))
        ctps = es.enter_context(tc.tile_pool(name="ctps", bufs=2, space="PSUM"))
        tps4 = es.enter_context(tc.tile_pool(name="tps4", bufs=2, space="PSUM"))

        # ---- tile allocation (no DMAs yet; DMA order is controlled below)
        ident = consts.tile([P, P], F32)
        ident16 = consts.tile([P, P], DT)
        ones_f32 = consts.tile([1, P], F32)
        ones_row = consts.tile([1, P], DT)
        ones64 = consts.tile([P, 64], F32)
        mask_sb = consts.tile([P, 16], F32)
        bq_sb = consts.tile([P, 2], F32)
        bk_sb = consts.tile([P, 2], F32)
        bv0 = consts.tile([1, DC], F32)
        bv16 = consts.tile([1, DC], DT)
        bvfull = consts.tile([P, DC], F32)

        XT = qkvp.tile([P, 8, L], DT, tag="xt")
        WT = {
            n: wtp.tile([P, 8, DC], DT, tag=f"wt{n}", name=f"wt{n}")
            for n in ("q", "k", "v")
        }
        QT = [qkvp.tile([P, L], DT, tag=f"qt{j}", name=f"qt{j}") for j in range(2)]
        KT = [qkvp.tile([P, L], DT, tag=f"kt{j}", name=f"kt{j}") for j in range(2)]
        Vt = qkvp.tile([P, 16, 4, 65], DT, tag="v")

        # 128-token chunks; chunks 0/2 arrive f32 on SP (fast ramp), the
        # rest fp16 via gpsimd casting DMA (cheaper transposes + copies).
        # Slot generations follow writer EMISSION order (monotonic below).
        F32_CHUNKS = (0, 2, 4, 6, 8, 10, 12, 14)  # f32 on SP; odd fp16 on gpsimd
        _fidx = {}
        for c in range(16):
            if c not in F32_CHUNKS:
                _fidx[c] = len(_fidx)
        xns = [
            xnat.tile(
                [P, HF],
                F32 if c in F32_CHUNKS else DT,
                tag=(
                    f"xe{(c // 2) % 3}"
                    if c in F32_CHUNKS
                    else f"xf{_fidx[c] % 4}"
                ),
                name=f"xn{c}",
            )
            for c in range(16)
        ]
        wns = {
            n: wnat.tile([P, 2, HF], DT, tag=f"wn{n}", name=f"wn{n}")
            for n in ("q", "k", "v")
        }

        # ---- const init first: the identity must precede the W DMAs in the
        # gpsimd stream or the first PE transposes wait ~5us
        make_identity(nc, ident)
        nc.gpsimd.memset(ones_f32, 1.0)
        nc.gpsimd.memset(ones64, 1.0)
        nc.vector.tensor_copy(ident16, ident)
        nc.vector.tensor_copy(ones_row, ones_f32)
        nc.vector.tensor_copy(
            Vt[:, :, :, 64], ones64.rearrange("p (t h) -> p t h", h=4)
        )

        # ---- DMA issue order (per engine queue = transfer order)
        def _xdma(eng, c):
            eng.dma_start(xns[c], x[ds(128 * c, 128), :])

        _xdma(nc.sync, 0)
        # W as fp16 via gpsimd casting DMA: half the bytes, lands early
        nc.gpsimd.dma_start(wns["k"], wk.rearrange("(j p) i -> p j i", p=P))
        _xdma(nc.gpsimd, 1)
        _xdma(nc.sync, 2)
        _xdma(nc.gpsimd, 3)
        _xdma(nc.sync, 4)
        nc.gpsimd.dma_start(wns["q"], wq.rearrange("(j p) i -> p j i", p=P))
        nc.gpsimd.dma_start(wns["v"], wv.rearrange("(j p) i -> p j i", p=P))
        nc.sync.dma_start(bk_sb, bk.rearrange("(j p) -> p j", p=P))
        nc.sync.dma_start(bq_sb, bq.rearrange("(j p) -> p j", p=P))
        nc.sync.dma_start(mask_sb, mask.rearrange("(t p) -> p t", p=P))
        nc.sync.dma_start(bv0, bv[None, :])
        _xdma(nc.gpsimd, 5)
        _xdma(nc.sync, 6)
        for c in range(7, 16):
            if c in F32_CHUNKS:
                _xdma(nc.sync, c)
            else:
                _xdma(nc.gpsimd, c)
        nc.vector.tensor_copy(bv16, bv0)  # after the bv0 DMA is emitted

        # estimated DMA landing times (us) per the queue orders above
        dma_est = {
            ("w", "k"): 1.8,
            ("w", "q"): 8.2,
            ("w", "v"): 9.8,
            ("x", 0): 1.6,
            ("x", 1): 3.4,
            ("x", 2): 3.2,
            ("x", 3): 5.0,
            ("x", 4): 4.8,
            ("x", 5): 11.4,
            ("x", 6): 6.4,
            ("x", 7): 13.0,
            ("x", 8): 9.7,
            ("x", 9): 14.6,
            ("x", 10): 11.3,
            ("x", 11): 16.2,
            ("x", 12): 12.9,
            ("x", 13): 17.8,
            ("x", 14): 14.5,
            ("x", 15): 19.4,
        }
        GPSIMD_FREE = 19.5  # gpsimd engine busy with DMA transfers until then

        # ---- emission helpers
        def _cp(eng, dst, src):
            if eng is nc.scalar:
                eng.copy(dst, src)
            else:
                eng.tensor_copy(dst, src)

        def _psum_flat(n_f32):
            """One tps4 bank slot viewed as a flat [P, n_f32] f32 ap."""
            t_ = tps4.tile([P, 4, P], F32, tag="tp4")
            return t_[:, :, :].rearrange("p a b -> p (a b)")[:, 0:n_f32]

        def emit_wt(name, engs):
            if name == "v":
                # broadcast bias tile: ones-column x bias-row, built once
                ps = _psum_flat(DC)
                nc.tensor.matmul(
                    ps, ones_row[0:1, :], bv16[0:1, :], start=True, stop=True
                )
                nc.vector.tensor_copy(bvfull, ps)
            # fp16 source: transpose blocks into the f32r staging tile via a
            # bitcast view (fp16 identity, 1 cycle/row)
            wn = wns[name]
            for jj in range(2):
                for k0 in (0, 4):
                    t_ = tps4.tile([P, 4, P], F32, tag="tp4")
                    v_ = t_[:, :, :].bitcast(DT)  # [P, 4, 256] view
                    for i in range(4):
                        nc.tensor.transpose(
                            v_[:, i, 0:P], wn[:, jj, ts(k0 + i, P)], ident16
                        )
                    dst = WT[name][:, :, ts(jj, P)]
                    if len(engs) == 2:
                        _cp(engs[0], dst[:, k0 : k0 + 2, :], v_[:, 0:2, 0:P])
                        _cp(engs[1], dst[:, k0 + 2 : k0 + 4, :], v_[:, 2:4, 0:P])
                    else:
                        _cp(engs[0], dst[:, k0 : k0 + 4, :], v_[:, :, 0:P])

        def emit_xt(ch, engs):  # 128-token chunk 0..15
            xn = xns[ch]
            fp16 = ch not in F32_CHUNKS
            for k0 in (0, 4):
                t_ = tps4.tile([P, 4, P], F32, tag="tp4")
                if fp16:
                    v_ = t_[:, :, :].bitcast(DT)  # [P, 4, 256] view
                    for i in range(4):
                        nc.tensor.transpose(
                            v_[:, i, 0:P], xn[:, ts(k0 + i, P)], ident16
                        )
                    src = [v_[:, 0:2, 0:P], v_[:, 2:4, 0:P], v_[:, :, 0:P]]
                else:
                    for i in range(4):
                        nc.tensor.transpose(
                            t_[:, i, :], xn[:, ts(k0 + i, P)], ident
                        )
                    src = [t_[:, 0:2, :], t_[:, 2:4, :], t_[:, :, :]]
                dst = XT[:, :, ts(ch, P)]
                if len(engs) == 2:
                    _cp(engs[0], dst[:, k0 : k0 + 2, :], src[0])
                    _cp(engs[1], dst[:, k0 + 2 : k0 + 4, :], src[1])
                else:
                    _cp(engs[0], dst[:, k0 : k0 + 4, :], src[2])

        def kq_proj(name, Tarr, b_sb, qc, jj):  # 512-token chunk 0..3, head-pair jj
            ps = _psum_flat(512)
            for ic in range(8):
                nc.tensor.matmul(
                    ps,
                    WT[name][:, ic, ts(jj, P)],
                    XT[:, ic, ts(qc, 512)],
                    start=(ic == 0),
                    stop=(ic == 7),
                )
            nc.vector.tensor_scalar_add(
                Tarr[jj][:, ts(qc, 512)], ps, b_sb[:, jj : jj + 1]
            )

        def v_proj(tt):  # token tile 0..15
            ps = _psum_flat(DC)
            for ic in range(8):
                nc.tensor.matmul(
                    ps,
                    XT[:, ic, ts(tt, P)],
                    WT["v"][:, ic, :],
                    start=(ic == 0),
                    stop=(ic == 7),
                )
            nc.vector.tensor_tensor(
                Vt[:, tt, :, 0:64],
                ps.rearrange("p (h c) -> p h c", c=64),
                bvfull.rearrange("p (h c) -> p h c", c=64),
                mybir.AluOpType.add,
            )

        pts_state = {}  # (qb, j) -> {kc: pt}
        csh_state = {}  # (qb, j) -> [cs accumulator per hl]
        out_tiles = {}

        def attn_S(qb, j, b):  # kc batch b: kc 4b..4b+3
            pts = pts_state.setdefault((qb, j), {})
            for kc in range(4 * b, 4 * b + 4):
                stt = bigp.tile([P, 1024], F32, tag="big")
                nc.tensor.matmul(
                    stt[:, 0:512],
                    KT[j][0:64, ts(kc, P)],
                    QT[j][0:64, ts(qb, 512)],
                    start=True,
                    stop=True,
                    tile_position=(0, 0),
                )
                nc.tensor.matmul(
                    stt[:, 512:1024],
                    KT[j][64:128, ts(kc, P)],
                    QT[j][64:128, ts(qb, 512)],
                    start=True,
                    stop=True,
                    tile_position=(64, 0),
                )
                pt = ptp.tile([P, 1024], DT, tag="pt")
                nc.scalar.activation(
                    pt, stt, EXP, bias=mask_sb[:, kc : kc + 1], scale=0.125
                )
                pts[kc] = pt

        def attn_C(qb, j, b):
            """C half-burst for kc batch pair b (8 chunks): accumulate in
            psum, then copy (b==0) or add (b==1) into the SBUF accumulator."""
            pts = pts_state[(qb, j)]
            kcs = range(8 * b, 8 * b + 8)
            CT = [
                ctps.tile([65, 512], F32, tag="ct", name=f"ct{qb}_{j}_{b}_{hl}")
                for hl in range(2)
            ]
            for kc in kcs:
                for hl in range(2):
                    nc.tensor.matmul(
                        CT[hl],
                        Vt[:, kc, 2 * j + hl, :],
                        pts[kc][:, ts(hl, 512)],
                        start=(kc % 8 == 0),
                        stop=(kc % 8 == 7),
                    )
            for kc in kcs:
                del pts[kc]
            if b == 0:
                csh_state[(qb, j)] = csh = []
                for hl in range(2):
                    cs = csp.tile([65, 512], F32, tag=f"cs{hl}", name=f"cs{hl}")
                    nc.vector.tensor_copy(cs, CT[hl])
                    csh.append(cs)
            else:
                csh = csh_state[(qb, j)]
                for hl in range(2):
                    nc.vector.tensor_tensor(
                        csh[hl], csh[hl], CT[hl], mybir.AluOpType.add
                    )

        def drain(qb, j):
            del pts_state[(qb, j)]
            csh = csh_state.pop((qb, j))
            if qb not in out_tiles:
                out_tiles[qb] = outp.tile([P, 4, DC], F32, tag="out", name=f"o{qb}")
            OUT = out_tiles[qb]
            for hl in range(2):
                h = 2 * j + hl
                tp = tps4.tile([P, 4, P], F32, tag="tp4")
                for cc in range(4):
                    nc.tensor.transpose(
                        tp[:, cc, 0:65],
                        csh[hl][:, ts(cc, P)],
                        ident[0:65, 0:65],
                    )
                tpb = drp.tile([P, 4, 65], F32, tag="tpb")
                nc.vector.tensor_copy(tpb, tp[:, :, 0:65])
                rcp = rcpp.tile([P, 4], F32, tag="rcp")
                nc.vector.reciprocal(
                    rcp, tpb[:, :, 64:65].rearrange("p a b -> p (a b)")
                )
                for cc in range(4):
                    nc.gpsimd.tensor_scalar_mul(
                        OUT[:, cc, ts(h, 64)], tpb[:, cc, 0:64], rcp[:, cc : cc + 1]
                    )
            nc.sync.dma_start(
                out[ds(512 * qb, 512), ds(128 * j, 128)].rearrange(
                    "(c p) d -> p c d", p=P
                ),
                OUT[:, :, ts(j, P)],
            )
            if j == 1:
                out_tiles.pop(qb)

        # ---- greedy static scheduler ------------------------------------
        # Tracks a coarse PE-time estimate and the ACT (exp) frontier and
        # chooses, at each step, between feeding the exp stream (S batch),
        # unblocking future tiers (transposes + K/Q projections), retiring
        # exp outputs (C bursts, bounded by the pt ring), V projections and
        # drains.
        PASSES = [(qb, j) for qb in range(4) for j in range(2)]
        S_PE, S_ACT = 1.75, 4.22  # us per 4-kc batch
        COST = {"xt": 0.7, "wt": 0.95, "kq": 1.85, "v": 1.05, "C": 3.5, "dr": 0.6}

        done = set()  # emitted wt/xt/kq/v items
        N_B_ITEMS = 3 + 16 + 16 + 16  # wt, xt, kq(per jj), v
        next_b = {p: 0 for p in PASSES}  # next S batch per pass
        c_done = {p: 0 for p in PASSES}  # C halves emitted per pass
        drained = set()
        pts_out = 0
        pe_t = 0.0  # PE frontier estimate (us)
        act_t = 0.0  # ACT frontier estimate
        act_started = False

        def engs_now():
            # gpsimd cannot access PSUM, so transpose copybacks go to DVE
            return (nc.vector,)

        def s_ready(p):
            b = next_b[p]
            if b > 3:
                return False
            qb, j = p
            return ("kq", "k", b, j) in done and ("kq", "q", qb, j) in done

        def c_ready(p):
            h = c_done[p]
            if h > 1:
                return False
            if h == 0:
                # the cs accumulator ring holds 3 passes: don't open a fourth
                n_open = sum(
                    1 for q in PASSES if c_done[q] >= 1 and q not in drained
                )
                if n_open >= 3:
                    return False
            if next_b[p] < 2 * (h + 1):  # both S batches of the half emitted
                return False
            return all(("v", tt) in done for tt in range(8 * h, 8 * h + 8))

        def emit(item):
            nonlocal pe_t, act_t, act_started, pts_out
            kind = item[0]
            if kind == "wt":
                pe_t = max(pe_t, dma_est[("w", item[1])]) + COST["wt"]
                emit_wt(item[1], engs_now())
                done.add(item)
            elif kind == "xt":
                pe_t = max(pe_t, dma_est[("x", item[1])]) + COST["xt"]
                emit_xt(item[1], engs_now())
                done.add(item)
            elif kind == "kq":
                _, name, qc, jj = item
                pe_t += COST["kq"]
                if name == "k":
                    kq_proj("k", KT, bk_sb, qc, jj)
                else:
                    kq_proj("q", QT, bq_sb, qc, jj)
                done.add(item)
            elif kind == "v":
                pe_t += COST["v"]
                v_proj(item[1])
                done.add(item)
            elif kind == "S":
                _, p = item
                pe_t += S_PE
                act_t = max(act_t, pe_t + 0.3) + S_ACT
                act_started = True
                attn_S(p[0], p[1], next_b[p])
                next_b[p] += 1
                pts_out += 4
            elif kind == "C":
                _, p = item
                pe_t += COST["C"]
                attn_C(p[0], p[1], c_done[p])
                c_done[p] += 1
                pts_out -= 8
            elif kind == "dr":
                _, p = item
                pe_t += COST["dr"]
                drain(p[0], p[1])
                drained.add(p)

        kq_order = [
            ("kq", n, qc, jj)
            for qc in range(4)
            for jj in range(2)
            for n in ("k", "q")
        ]
        v_next = 0

        def pick_b():
            nonlocal v_next
            # 1. K/Q projection chains (weights -> X transposes -> proj), in
            #    dependency order; skip chains whose DMA hasn't landed
            for it in kq_order:
                if it in done:
                    continue
                _, n, qc, jj = it
                wt_it = ("wt", n)
                if wt_it not in done:
                    if dma_est[("w", n)] <= pe_t + 0.6:
                        return wt_it
                    continue
                need = [("xt", 4 * qc + i) for i in range(4)]
                missing = [m for m in need if m not in done]
                if missing:
                    m = missing[0]
                    if dma_est[("x", m[1])] <= pe_t + 0.6:
                        return m
                    continue
                return it
            # 2. V projections (unlock C bursts)
            if v_next < 16:
                wt_it = ("wt", "v")
                if wt_it not in done:
                    if dma_est[("w", "v")] <= pe_t + 0.6:
                        return wt_it
                elif ("xt", v_next) in done:
                    tt = v_next
                    v_next += 1
                    return ("v", tt)
            # 3. C bursts (close open passes first) / drains
            for p in sorted(PASSES, key=lambda q: -c_done[q]):
                if c_ready(p):
                    return ("C", p)
            for p in PASSES:
                if p not in drained and c_done[p] == 2:
                    if p[1] == 1 and (p[0], 0) not in drained:
                        continue
                    return ("dr", p)
            # 4. leftover X transposes
            for c in range(16):
                it = ("xt", c)
                if it not in done and dma_est[("x", c)] <= pe_t + 0.6:
                    return it
            return None

        emitted_S = 0
        guard = 0
        while (
            emitted_S < 32
            or len(done) < N_B_ITEMS
            or any(c_done[p] < 2 for p in PASSES)
            or len(drained) < 8
        ):
            guard += 1
            assert guard < 20000, "scheduler wedged"
            # feed the exp stream first whenever it is close to starving.
            # Opening a new half (even batch) reserves ring room for its
            # closing batch, so a full ring can always be retired.
            if act_t - pe_t < 4.0:
                sp = next(
                    (
                        p
                        for p in PASSES
                        if s_ready(p)
                        and pts_out
                        <= PT_RING - (12 if next_b[p] % 2 == 0 else 4)
                    ),
                    None,
                )
                if sp is not None:
                    emit(("S", sp))
                    emitted_S += 1
                    continue
            # retire exp outputs if the ring is filling up
            if pts_out >= 16:
                cp_ = next(
                    (
                        p
                        for p in sorted(PASSES, key=lambda q: -c_done[q])
                        if c_ready(p)
                    ),
                    None,
                )
                if cp_ is not None:
                    emit(("C", cp_))
                    continue
                # no C ready: the blocker is V (or Wv) -- pump it now
                if v_next < 16:
                    if ("wt", "v") not in done:
                        if dma_est[("w", "v")] <= pe_t + 0.6:
                            emit(("wt", "v"))
                            continue
                    elif ("xt", v_next) in done:
                        emit(("v", v_next))
                        v_next += 1
                        continue
            it = pick_b()
            if it is not None:
                emit(it)
                continue
            # nothing dep-ready: emit any ready S even if ACT is far ahead
            sp = next(
                (
                    p
                    for p in PASSES
                    if s_ready(p)
                    and pts_out <= PT_RING - (12 if next_b[p] % 2 == 0 else 4)
                ),
                None,
            )
            if sp is not None:
                emit(("S", sp))
                emitted_S += 1
                continue
            # last resort: advance the DMA clock (early ramp)
            pe_t += 0.4


def build_program(repeat=1, phases="all", loop=False):
    nc = bacc.Bacc("TRN2")
    x = nc.dram_tensor("x", [L, HF], F32, kind="ExternalInput").ap()
    wq = nc.dram_tensor("wq", [DC, HF], F32, kind="ExternalInput").ap()
    wk = nc.dram_tensor("wk", [DC, HF], F32, kind="ExternalInput").ap()
    wv = nc.dram_tensor("wv", [DC, HF], F32, kind="ExternalInput").ap()
    bq = nc.dram_tensor("bq", [DC], F32, kind="ExternalInput").ap()
    bk = nc.dram_tensor("bk", [DC], F32, kind="ExternalInput").ap()
    bv = nc.dram_tensor("bv", [DC], F32, kind="ExternalInput").ap()
    mask = nc.dram_tensor("mask", [L], F32, kind="ExternalInput").ap()
    out = nc.dram_tensor("out", [L, DC], F32, kind="ExternalOutput").ap()
    with tile.TileContext(nc) as tc:
        if loop and repeat > 1:
            with tc.For_i(0, repeat, 1):
                _emit(tc, x, wq, wk, wv, bq, bk, bv, mask, out, phases=phases)
        else:
            for _rep in range(repeat):
                _emit(tc, x, wq, wk, wv, bq, bk, bv, mask, out, phases=phases)
    nc.compile()
    return nc


_PROGS = {}


def _get_prog(repeat=1, phases="all", loop=False):
    key = (repeat, phases, loop)
    if key not in _PROGS:
        _PROGS[key] = build_program(repeat, phases, loop)
    return _PROGS[key]


def make_in_maps(hidden_states, attention_mask, Wq, bq, Wk, bk, Wv, bv):
    hs = np.ascontiguousarray(np.asarray(hidden_states, dtype=np.float32))
    am = np.asarray(attention_mask, dtype=np.float32)
    Wq, Wk, Wv = (np.asarray(w, dtype=np.float32) for w in (Wq, Wk, Wv))
    bq, bk, bv = (np.asarray(b, dtype=np.float32) for b in (bq, bk, bv))
    in_maps = []
    for c in range(8):
        b, g = divmod(c, 4)
        sl = slice(DC * g, DC * (g + 1))
        in_maps.append(
            {
                "x": hs[b],
                "wq": np.ascontiguousarray(Wq[sl]),
                "wk": np.ascontiguousarray(Wk[sl]),
                "wv": np.ascontiguousarray(Wv[sl]),
                "bq": np.ascontiguousarray(bq[sl]),
                "bk": np.ascontiguousarray(bk[sl]),
                "bv": np.ascontiguousarray(bv[sl]),
                "mask": np.ascontiguousarray(am[b, 0, 0, :]),
            }
        )
    return in_maps


def run_cores(in_maps, trace=False, **kw):
    nc = _get_prog()
    return run_bass_kernel_spmd(nc, in_maps, list(range(8)), trace=trace, **kw)


def assemble(results):
    out = np.empty((2, L, HF), dtype=np.float32)
    for c in range(8):
        b, g = divmod(c, 4)
        out[b, :, DC * g : DC * (g + 1)] = results[c]["out"]
    return out


def kernel(hidden_states, attention_mask, Wq, bq, Wk, bk, Wv, bv):
    in_maps = make_in_maps(hidden_states, attention_mask, Wq, bq, Wk, bk, Wv, bv)
    res = run_cores(in_maps)
    return assemble(res.results)
